# revision 60
# baseline (speedup 1.0000x reference)
"""Trainium2 Bass kernel for ClassifierConvLSTM1D.

Model (hardcoded shapes): x[64,1536,512] -> AvgPool1D(6) -> dense gates
GEMM (W[512,1024]) -> 256-step LSTM recurrence (R[256,1024], hard_sigmoid
i/f/o gates, tanh g) -> dense head (Wd[256,250]) -> softmax.

Strategy: data-parallel over batch across 8 NeuronCores (8 samples/core,
weights replicated). Per core:
  Phase A: stream x, fuse avg-pool + transpose into PE matmuls against a
           pooling matrix, then GEMM1 with W stationary -> zx^T in SBUF
           laid out [128 gate-partitions, (m-chunk, batch), time].
  Phase B: 256 fully-unrolled LSTM steps. R stationary on PE, states kept
           transposed [unit%128, (unit//128, batch)] so gate elementwise
           ops are [128, 16..48] tiles. hard_sigmoid is folded to
           add+clip by pre-scaling W/R/b columns of i,f,o by 0.2 (+0.5
           bias folded into zx).
  Head: logits via h^T-stationary matmuls (+bias via K=1 matmul with a
        ones vector), softmax along the free dim.
No collectives needed; outputs gathered host-side.
"""

import sys

if "/opt/trn_rl_repo" not in sys.path:
    sys.path.insert(0, "/opt/trn_rl_repo")

from contextlib import ExitStack

import numpy as np

import concourse.bass as bass  # noqa: F401  (registers AP helpers)
import concourse.tile as tile
from concourse import bacc, mybir
from concourse.bass_utils import run_bass_kernel_spmd
from concourse.masks import make_identity

B, T, F = 64, 1536, 512
POOL, UNITS, NCLS = 6, 256, 250
TP = T // POOL  # 256
G = 4 * UNITS  # 1024
NCORES = 8
BC = B // NCORES  # 8 samples per core

F32 = mybir.dt.float32
F16 = mybir.dt.float16
AF = mybir.ActivationFunctionType
ALU = mybir.AluOpType

_CACHE: dict = {}


def _build_program():
    nc = bacc.Bacc(
        "TRN2",
        debug=False,
        enable_asserts=False,
        num_devices=NCORES,
    )

    x_d = nc.dram_tensor("x", [BC, T, F], F32, kind="ExternalInput").ap()
    wl_d = nc.dram_tensor("wl", [128, 4 * 8 * 128], F16, kind="ExternalInput").ap()
    rl_d = nc.dram_tensor("rl", [128, 2 * 8 * 128], F16, kind="ExternalInput").ap()
    bias_d = nc.dram_tensor("bias", [128, 8], F32, kind="ExternalInput").ap()
    wdl_d = nc.dram_tensor("wdl", [128, 2 * NCLS], F32, kind="ExternalInput").ap()
    bdl_d = nc.dram_tensor("bdl", [1, NCLS], F32, kind="ExternalInput").ap()
    p6_d = nc.dram_tensor("p6", [126, 21], F32, kind="ExternalInput").ap()
    p6b_d = nc.dram_tensor("p6b", [24, 4], F32, kind="ExternalInput").ap()
    out_d = nc.dram_tensor("out", [BC, NCLS], F32, kind="ExternalOutput").ap()

    with tile.TileContext(nc) as tc, ExitStack() as ctx:
        cpool = ctx.enter_context(tc.tile_pool(name="const", bufs=1))
        w_sb = cpool.tile([128, 4 * 8 * 128], F16)
        nc.sync.dma_start(w_sb[:], wl_d)
        r_sb = cpool.tile([128, 2 * 8 * 128], F16)
        nc.sync.dma_start(r_sb[:], rl_d)
        ident = cpool.tile([128, 128], F32)
        make_identity(nc, ident[:])
        bias_sb = cpool.tile([128, 8], F32)
        nc.sync.dma_start(bias_sb[:], bias_d)
        wd_sb = cpool.tile([128, 2 * NCLS], F32)
        nc.sync.dma_start(wd_sb[:], wdl_d)
        bd_sb = cpool.tile([1, NCLS], F32)
        nc.sync.dma_start(bd_sb[:], bdl_d)
        p6_sb = cpool.tile([126, 21], F32)
        nc.sync.dma_start(p6_sb[:], p6_d)
        p6b_sb = cpool.tile([24, 4], F32)
        nc.sync.dma_start(p6b_sb[:], p6b_d)
        ones_sb = cpool.tile([1, 8], F32)
        nc.vector.memset(ones_sb[:], 1.0)

        zx_pool = ctx.enter_context(tc.tile_pool(name="zx", bufs=1))
        # [gate%128, (m-chunk*8 + batch), pooled-time]; m-chunks 0-5 = i,f,o
        # (pre-scaled for hard_sigmoid), 6-7 = g. Two time-halves (block-
        # aligned: 6 pool-blocks = 126 tp, then the rest) so the recurrence
        # starts after only half of x has streamed; the second half of
        # phase A hides under the running recurrence.
        NH0 = 126
        NHS = [NH0, TP - NH0]
        zxT_h = [
            zx_pool.tile([128, 64, NHS[h]], F32, name=f"zxT{h}")
            for h in range(2)
        ]

        # ---------------- Phase A: pool + transpose + GEMM1 ----------------
        with ExitStack() as actx:
            xin_pool = actx.enter_context(tc.tile_pool(name="xin", bufs=6))
            xpt_pool = actx.enter_context(tc.tile_pool(name="xpt", bufs=8))
            pp_pool = actx.enter_context(
                tc.tile_pool(name="pp", bufs=4, space="PSUM")
            )
            zp_pool = actx.enter_context(
                tc.tile_pool(name="zp", bufs=4, space="PSUM")
            )
            def gemm_half(hh, bb):
                for m in range(8):
                    zp = zp_pool.tile(
                        [128, NHS[hh]], F32, tag="zp", name=f"zp_{hh}_{bb}_{m}"
                    )
                    for kc in range(4):
                        nc.tensor.matmul(
                            zp[:],
                            w_sb[:, (kc * 8 + m) * 128 : (kc * 8 + m + 1) * 128],
                            xpts[bb][:, kc, hh * NH0 : hh * NH0 + NHS[hh]],
                            start=(kc == 0),
                            stop=(kc == 3),
                        )
                    # biased copy PSUM -> SBUF (adds 0.2*b+0.5 / b per gate)
                    nc.vector.tensor_scalar(
                        zxT_h[hh][:, m * 8 + bb, :], zp[:],
                        bias_sb[:, m : m + 1], None, ALU.add,
                    )

            dma_engines = [nc.sync, nc.scalar]
            xpts = [
                xpt_pool.tile([128, 4, TP], F16, tag="xpt", name=f"xpt{b}")
                for b in range(BC)
            ]

            def pool_blocks(bb, src6, blk0):
                # pool 6 blocks of 126 timesteps from one [126, 6, F] tile
                for j in range(6):
                    blk = blk0 + j
                    po = blk * 21
                    pp = pp_pool.tile(
                        [128, 4, 32], F32, tag="pp", name=f"pp_{bb}_{blk}"
                    )
                    for kc in range(4):
                        nc.tensor.matmul(
                            pp[:, kc, :21],
                            src6[:, j, kc * 128 : (kc + 1) * 128],
                            p6_sb[:, :],
                            start=True,
                            stop=True,
                        )
                    nc.vector.tensor_copy(
                        xpts[bb][:, :, po : po + 21], pp[:, :, :21]
                    )

            # ---- first half of x: stream, pool, GEMM; unblocks recurrence
            for bb in range(BC):
                xt = xin_pool.tile([126, 6, F], F32, tag="xt", bufs=4)
                dma_engines[bb % 2].dma_start(
                    xt[:],
                    x_d[bb, 0:756, :].rearrange("(blk t) f -> t blk f", t=126),
                )
                pool_blocks(bb, xt, 0)
                gemm_half(0, bb)
            # ---- second half streams under the running recurrence
            for bb in range(BC):
                xt = xin_pool.tile([126, 6, F], F32, tag="xt", bufs=4)
                dma_engines[bb % 2].dma_start(
                    xt[:],
                    x_d[bb, 756:1512, :].rearrange("(blk t) f -> t blk f", t=126),
                )
                xtt = xin_pool.tile([24, F], F32, tag="xtt")
                dma_engines[bb % 2].dma_start(xtt[:], x_d[bb, 1512:1536, :])
                pool_blocks(bb, xt, 6)
                # tail block: 4 pooled steps
                pp = pp_pool.tile([128, 4, 32], F32, tag="pp", name=f"ppt_{bb}")
                for kc in range(4):
                    nc.tensor.matmul(
                        pp[:, kc, :4],
                        xtt[:24, kc * 128 : (kc + 1) * 128],
                        p6b_sb[:, :],
                        start=True,
                        stop=True,
                    )
                nc.vector.tensor_copy(
                    xpts[bb][:, :, 252:256], pp[:, :, :4]
                )
                gemm_half(1, bb)

        # ---------------- Phase B: LSTM recurrence (unrolled) ----------------
        nc._phase_markers = getattr(nc, "_phase_markers", {})
        nc._phase_markers["recur_start"] = len(nc.inst_map)
        st_pool = ctx.enter_context(tc.tile_pool(name="state", bufs=2))

        # each gate group gets its own PSUM bank so its consumer starts as
        # soon as its own 4 matmuls are done (deps are bank-level)
        GSLC = {"g": (6, 48), "f": (2, 16), "i": (0, 0), "o": (4, 32)}

        h_prev = st_pool.tile([128, 16], F16, tag="h", name="h_init")
        nc.vector.memset(h_prev[:], 0.0)
        c_prev = st_pool.tile([128, 16], F32, tag="c", name="c_init")
        nc.vector.memset(c_prev[:], 0.0)

        with ExitStack() as bctx:
            g_pool = bctx.enter_context(tc.tile_pool(name="gates", bufs=2))
            zps_pool = bctx.enter_context(
                tc.tile_pool(name="zps", bufs=2, space="PSUM")
            )

            for t in range(TP):
                hh = 0 if t < NH0 else 1
                zxt = zxT_h[hh][:, :, t - hh * NH0]
                # zx prefill idMMs are h-independent: they run during the
                # previous step's gate chain, off the critical path.
                # f and o share one PSUM bank (both readers are on DVE).
                zp = {}
                for gk in ("g", "f", "i", "o"):
                    m0, c0 = GSLC[gk]
                    zp[gk] = zps_pool.tile(
                        [128, 16], F32, tag=f"zps{gk}", name=f"zp_{gk}_{t}"
                    )
                    nc.tensor.matmul(
                        zp[gk][:], ident[:], zxt[:, c0 : c0 + 16],
                        start=True, stop=False,
                    )
                # recurrent matmuls in g,i,f,o order: each group's PSUM bank
                # completes early so its consumer chain starts while later
                # groups are still on the PE
                for gk in ("g", "i", "f", "o"):
                    m0, _ = GSLC[gk]
                    for m in (m0, m0 + 1):
                        for kc in range(2):
                            nc.tensor.matmul(
                                zp[gk][:, (m - m0) * 8 : (m - m0 + 1) * 8],
                                r_sb[:, (kc * 8 + m) * 128 : (kc * 8 + m + 1) * 128],
                                h_prev[:, kc * 8 : (kc + 1) * 8],
                                start=False,
                                stop=(m == m0 + 1 and kc == 1),
                                skip_group_check=True,
                            )
                gt = g_pool.tile([128, 16], F32, tag="gt")
                nc.scalar.activation(gt[:], zp["g"][:], AF.Tanh)
                iclip = g_pool.tile([128, 16], F32, tag="iclip")
                nc.vector.tensor_scalar(
                    iclip[:], zp["i"][:], 0.0, 1.0, ALU.max, ALU.min
                )
                fclip = g_pool.tile([128, 16], F32, tag="fclip")
                nc.vector.tensor_scalar(
                    fclip[:], zp["f"][:], 0.0, 1.0, ALU.max, ALU.min
                )
                cf = g_pool.tile([128, 16], F32, tag="cf")
                nc.vector.tensor_mul(cf[:], fclip[:], c_prev[:])
                ig = g_pool.tile([128, 16], F32, tag="ig")
                ig_i = nc.vector.tensor_mul(ig[:], iclip[:], gt[:])
                c_new = st_pool.tile([128, 16], F32, tag="c")
                nc.vector.tensor_add(c_new[:], ig[:], cf[:])
                th = g_pool.tile([128, 16], F32, tag="th")
                nc.scalar.activation(th[:], c_new[:], AF.Tanh)
                oclip = g_pool.tile([128, 16], F32, tag="oclip")
                oc_i = nc.vector.tensor_scalar(
                    oclip[:], zp["o"][:], 0.0, 1.0, ALU.max, ALU.min
                )
                tile.add_dep_helper(
                    ig_i.ins, oc_i.ins, sync=False,
                    reason="oclip off critical path",
                )
                h_new = st_pool.tile([128, 16], F16, tag="h")
                nc.vector.tensor_mul(h_new[:], oclip[:], th[:])
                h_prev, c_prev = h_new, c_new

            # fp32 copy of the final h for the head
            h_f32 = st_pool.tile([128, 16], F32, tag="hf")
            nc.vector.tensor_copy(h_f32[:], h_prev[:])
            h_prev = h_f32

        # ---------------- Head: logits + softmax ----------------
        nc._phase_markers["head_start"] = len(nc.inst_map)
        hd_pool = ctx.enter_context(tc.tile_pool(name="head", bufs=1))
        lp_pool = ctx.enter_context(tc.tile_pool(name="lp", bufs=1, space="PSUM"))
        lp = lp_pool.tile([BC, NCLS], F32)
        nc.tensor.matmul(
            lp[:], h_prev[:, 0:8], wd_sb[:, 0:NCLS], start=True, stop=False
        )
        nc.tensor.matmul(
            lp[:], h_prev[:, 8:16], wd_sb[:, NCLS : 2 * NCLS],
            start=False, stop=False,
        )
        nc.tensor.matmul(lp[:], ones_sb[:], bd_sb[:], start=False, stop=True)

        mx = hd_pool.tile([BC, 1], F32)
        nc.vector.reduce_max(mx[:], lp[:], axis=mybir.AxisListType.X)
        mxn = hd_pool.tile([BC, 1], F32)
        nc.vector.tensor_scalar_mul(mxn[:], mx[:], -1.0)
        e = hd_pool.tile([BC, NCLS], F32)
        s = hd_pool.tile([BC, 1], F32)
        nc.scalar.activation(e[:], lp[:], AF.Exp, bias=mxn[:], accum_out=s[:])
        rcp = hd_pool.tile([BC, 1], F32)
        nc.vector.reciprocal(rcp[:], s[:])
        o_sb = hd_pool.tile([BC, NCLS], F32)
        nc.vector.tensor_scalar(o_sb[:], e[:], rcp[:], None, ALU.mult)
        nc.sync.dma_start(out_d, o_sb[:])

    nc.compile()
    return nc


def _prep_weights(W, R, b, Wd, bd):
    # Keras gate order i,f,g,o -> reorder columns to i,f,o,g and pre-scale
    # the hard_sigmoid gates (i,f,o) by 0.2; fold the +0.5 into the bias.
    perm = np.concatenate(
        [np.arange(0, 256), np.arange(256, 512), np.arange(768, 1024),
         np.arange(512, 768)]
    )
    scale = np.ones(G, np.float32)
    scale[: 3 * UNITS] = 0.2
    shift = np.zeros(G, np.float32)
    shift[: 3 * UNITS] = 0.5

    Wp = (W[:, perm] * scale).astype(np.float32)
    Rp = (R[:, perm] * scale).astype(np.float32)
    bp = (b[perm] * scale + shift).astype(np.float32)

    wl = np.ascontiguousarray(
        Wp.reshape(4, 128, 8, 128).transpose(1, 0, 2, 3).reshape(128, 4096)
    ).astype(np.float16)
    rl = np.ascontiguousarray(
        Rp.reshape(2, 128, 8, 128).transpose(1, 0, 2, 3).reshape(128, 2048)
    ).astype(np.float16)
    bias = np.ascontiguousarray(bp.reshape(8, 128).T)
    wdl = np.ascontiguousarray(
        Wd.astype(np.float32).reshape(2, 128, NCLS).transpose(1, 0, 2).reshape(128, 2 * NCLS)
    )
    bdl = np.ascontiguousarray(bd.astype(np.float32).reshape(1, NCLS))

    p6 = np.zeros((126, 21), np.float32)
    p6[np.arange(126), np.arange(126) // 6] = 1.0 / 6.0
    p6b = np.zeros((24, 4), np.float32)
    p6b[np.arange(24), np.arange(24) // 6] = 1.0 / 6.0
    return wl, rl, bias, wdl, bdl, p6, p6b


def kernel(x, W, R, b, Wd, bd):
    x = np.asarray(x, np.float32)
    wl, rl, bias, wdl, bdl, p6, p6b = _prep_weights(
        np.asarray(W, np.float32), np.asarray(R, np.float32),
        np.asarray(b, np.float32), np.asarray(Wd, np.float32),
        np.asarray(bd, np.float32),
    )

    if "nc" not in _CACHE:
        _CACHE["nc"] = _build_program()
    nc = _CACHE["nc"]

    in_maps = []
    for i in range(NCORES):
        in_maps.append(
            {
                "x": np.ascontiguousarray(x[i * BC : (i + 1) * BC]),
                "wl": wl, "rl": rl, "bias": bias, "wdl": wdl, "bdl": bdl,
                "p6": p6, "p6b": p6b,
            }
        )
    res = run_bass_kernel_spmd(nc, in_maps, list(range(NCORES)))
    out = np.concatenate([res.results[i]["out"] for i in range(NCORES)], axis=0)
    return out.astype(np.float32)
